# revision 10
# baseline (speedup 1.0000x reference)
"""Trainium2 Bass kernel for nn_BITypeNetwork (16384-neuron BI-type network step).

Math: the reference computes, with adj/states exactly binary {0.0, 1.0},
    inter_i = 1 - prod_j (1 - adj[i,j] + adj[i,j]*states[j])
Each product term equals 1 - adj[i,j]*(1 - states[j]) which is 0 or 1, so
    inter_i = min(sum_j adj[i,j] * (1 - states[j]), 1)
i.e. a matvec against sp = 1 - states followed by a clamp — exact in fp32
(and the 0/1 operands are exact in bf16, halving the HBM traffic).
Tail:  out = 1 - (1 - c * roll(x, -1)) * inter.

Sharding: adj row-sharded across 8 cores (2048 rows each), sp broadcast,
c/x3 row-sharded. No cross-device reduction needed.

Per core the kernel streams its [2048, 16384] adj shard as bf16 through
DVE tensor_tensor multiply (2x_1p mode) with per-chunk row-sums taken by
ScalarE activation-accumulate / DVE tensor_scalar-accumulate (4x mode),
so both compute engines stay under the ~190us DMA roofline.
"""

import os
import sys

for _p in ("/opt/trn_rl_repo", "/opt/pypackages"):
    if os.path.isdir(_p) and _p not in sys.path:
        sys.path.insert(0, _p)

from contextlib import ExitStack

import ml_dtypes
import numpy as np

import concourse.bass as bass
import concourse.tile as tile
from concourse import bacc, mybir
from concourse.bass_utils import run_bass_kernel_spmd

N = 16384          # neurons
CORES = 8
R = N // CORES     # 2048 rows per core
P = 128            # SBUF partitions
T = R // P         # 16 row-tiles per core; local row = p*T + t
F = 8192           # free-dim chunk size
BF16 = mybir.dt.bfloat16
F32 = mybir.dt.float32

# Per-chunk compute style schedule:
#   "act": DVE tensor_tensor mult + ScalarE activation-accumulate reduce
#   "gps": DVE tensor_tensor mult + GpSimd tensor_scalar-accumulate reduce
#   "dve": DVE tensor_tensor mult + DVE tensor_scalar-accumulate reduce
#   "stt": fused DVE scalar_tensor_tensor multiply-accumulate (one op)
SCHEDULE = ["stt"] * 32


def _style(i):
    return SCHEDULE[i % len(SCHEDULE)]


def build_nc(n=N, r=R, f=F):
    t_tiles = r // P
    k_chunks = n // f
    nc = bacc.Bacc()
    adjb = nc.declare_dram_parameter("adjb", [r, n], BF16, isOutput=False)
    spb = nc.declare_dram_parameter("spb", [P, n], BF16, isOutput=False)
    cx_in = nc.declare_dram_parameter("cx", [2, r], F32, isOutput=False)
    out = nc.declare_dram_parameter("out", [r], F32, isOutput=True)

    adj_t = adjb.rearrange("(p t) n -> t p n", t=t_tiles)   # [T, 128, n]
    cx_t = cx_in.rearrange("v (p t) -> p v t", t=t_tiles)   # [128, 2, T]
    out_t = out.rearrange("(p t) -> p t", t=t_tiles)

    mult = mybir.AluOpType.mult
    add = mybir.AluOpType.add
    bypass = mybir.AluOpType.bypass

    with ExitStack() as ctx:
        tc = ctx.enter_context(tile.TileContext(nc))
        const = ctx.enter_context(tc.tile_pool(name="const", bufs=1))
        loadp = ctx.enter_context(tc.tile_pool(name="load", bufs=3))
        prodp = ctx.enter_context(tc.tile_pool(name="prod", bufs=2))
        sinkp = ctx.enter_context(tc.tile_pool(name="sink", bufs=2))
        partp = ctx.enter_context(tc.tile_pool(name="part", bufs=2))
        smallp = ctx.enter_context(tc.tile_pool(name="small", bufs=1))

        sp_tiles = []
        for k in range(k_chunks):
            spt = const.tile([P, f], BF16, tag=f"sp{k}")
            nc.sync.dma_start(spt[:], spb[:, bass.ts(k, f)])
            sp_tiles.append(spt)
        cx_tile = smallp.tile([P, 2, t_tiles], F32, tag="cx")
        nc.sync.dma_start(cx_tile[:], cx_t[:, :, :])
        d_tile = smallp.tile([P, t_tiles], F32, tag="d")

        # TRN2 allows at most one semaphore wait per instruction; touch each
        # sp tile with a tiny op so the DVE observes those DMA semaphores
        # one at a time before the main loop's tensor_tensor ops.
        touch = smallp.tile([P, 1], BF16, tag="touch")
        for k in range(k_chunks):
            nc.vector.tensor_copy(touch[:], sp_tiles[k][:, 0:1])

        i = 0
        for t in range(t_tiles):
            part = partp.tile([P, k_chunks], F32, tag="part")
            for k in range(k_chunks):
                a = loadp.tile([P, f], BF16, tag="adj")
                nc.sync.dma_start(a[:], adj_t[t][:, bass.ts(k, f)])
                style = _style(i)
                if style == "stt":
                    sink = sinkp.tile([P, f], BF16, tag="sink")
                    nc.vector.scalar_tensor_tensor(
                        sink[:], a[:], 1.0, sp_tiles[k][:],
                        op0=mult, op1=mult,
                        accum_out=part[:, k : k + 1],
                    )
                else:
                    prod = prodp.tile([P, f], BF16, tag="prod")
                    nc.vector.tensor_tensor(prod[:], a[:], sp_tiles[k][:], op=mult)
                    sink = sinkp.tile([P, f], BF16, tag="sink")
                    if style == "dve":
                        nc.vector.tensor_scalar(
                            sink[:], prod[:], 1.0, None,
                            op0=mult, op1=add,
                            accum_out=part[:, k : k + 1],
                        )
                    elif style == "gps":
                        nc.gpsimd.tensor_scalar(
                            sink[:], prod[:], 1.0, None,
                            op0=mult, op1=add,
                            accum_out=part[:, k : k + 1],
                        )
                    else:
                        nc.scalar.activation(
                            sink[:], prod[:],
                            mybir.ActivationFunctionType.Copy,
                            accum_out=part[:, k : k + 1],
                        )
                i += 1
            nc.vector.tensor_reduce(
                d_tile[:, t : t + 1], part[:], axis=mybir.AxisListType.X, op=add
            )

        # Epilogue on [128, T] fp32: out = 1 - (1 - c*x3) * min(d, 1)
        inter = smallp.tile([P, t_tiles], F32, tag="inter")
        nc.vector.tensor_scalar_min(inter[:], d_tile[:], 1.0)
        cn = smallp.tile([P, t_tiles], F32, tag="cn")
        nc.vector.tensor_tensor(cn[:], cx_tile[:, 0, :], cx_tile[:, 1, :], op=mult)
        nc.vector.tensor_scalar(cn[:], cn[:], -1.0, 1.0, op0=mult, op1=add)
        res = smallp.tile([P, t_tiles], F32, tag="res")
        nc.vector.tensor_tensor(res[:], cn[:], inter[:], op=mult)
        nc.vector.tensor_scalar(res[:], res[:], -1.0, 1.0, op0=mult, op1=add)
        nc.sync.dma_start(out_t[:, :], res[:])

    nc.compile()
    return nc


_NC_CACHE = None


def _get_nc():
    global _NC_CACHE
    if _NC_CACHE is None:
        _NC_CACHE = build_nc()
    return _NC_CACHE


def prep_in_maps(x, adj, states, c):
    x = np.asarray(x, dtype=np.float32).reshape(-1)
    adj = np.asarray(adj)
    states = np.asarray(states, dtype=np.float32).reshape(-1)
    c = np.asarray(c, dtype=np.float32).reshape(-1)

    adjb = adj.astype(ml_dtypes.bfloat16)          # exact: adj is 0/1
    sp = (1.0 - states).astype(ml_dtypes.bfloat16)  # exact: states is 0/1
    spb = np.ascontiguousarray(np.broadcast_to(sp[None, :], (P, N)))
    x3 = np.roll(x, -1)                             # x[(i+1) % N]

    in_maps = []
    for m in range(CORES):
        rows = slice(m * R, (m + 1) * R)
        in_maps.append(
            {
                "adjb": np.ascontiguousarray(adjb[rows]),
                "spb": spb,
                "cx": np.ascontiguousarray(np.stack([c[rows], x3[rows]])),
            }
        )
    return in_maps


def _ensure_ntff_hook():
    """Install antenv.axon_hooks shim so trace=True works under axon."""
    import types

    try:
        from antenv.axon_hooks import get_axon_ntff_profile_hook  # noqa: F401

        return
    except ImportError:
        pass
    import antenv
    from trn_agent_boot.trn_boot import _ntff_profile_via_ctypes

    hook = _ntff_profile_via_ctypes("/opt/axon/libaxon_pjrt.so")
    mod = types.ModuleType("antenv.axon_hooks")
    state = {"hook": hook}
    mod.set_axon_ntff_profile_hook = lambda h: state.__setitem__("hook", h)
    mod.get_axon_ntff_profile_hook = lambda: state["hook"]
    sys.modules["antenv.axon_hooks"] = mod
    antenv.axon_hooks = mod


def run(x, adj, states, c, trace=False, **kw):
    if trace:
        _ensure_ntff_hook()
    in_maps = prep_in_maps(x, adj, states, c)
    res = run_bass_kernel_spmd(
        _get_nc(), in_maps, list(range(CORES)), trace=trace, **kw
    )
    outs = [np.asarray(res.results[m]["out"], dtype=np.float32) for m in range(CORES)]
    full = np.concatenate([o.reshape(R) for o in outs])
    return full, res


def kernel(x, adj, states, c):
    full, _ = run(x, adj, states, c)
    return full


# revision 12
# speedup vs baseline: 1.2007x; 1.2007x over previous
"""Trainium2 Bass kernel for nn_BITypeNetwork (16384-neuron BI-type network step).

Math: the reference computes, with adj/states exactly binary {0.0, 1.0},
    inter_i = 1 - prod_j (1 - adj[i,j] + adj[i,j]*states[j])
Each product term equals 1 - adj[i,j]*(1 - states[j]) which is 0 or 1, so
    inter_i = min(sum_j adj[i,j] * (1 - states[j]), 1)
i.e. a matvec against sp = 1 - states followed by a clamp — exact in fp32
(and the 0/1 operands are exact in bf16, halving the HBM traffic).
Tail:  out = 1 - (1 - c * roll(x, -1)) * inter.

Sharding: adj row-sharded across 8 cores (2048 rows each), sp broadcast,
c/x3 row-sharded. No cross-device reduction needed.

Per core the kernel streams its [2048, 16384] adj shard as bf16 through
DVE tensor_tensor multiply (2x_1p mode) with per-chunk row-sums taken by
ScalarE activation-accumulate / DVE tensor_scalar-accumulate (4x mode),
so both compute engines stay under the ~190us DMA roofline.
"""

import os
import sys

for _p in ("/opt/trn_rl_repo", "/opt/pypackages"):
    if os.path.isdir(_p) and _p not in sys.path:
        sys.path.insert(0, _p)

from contextlib import ExitStack

import ml_dtypes
import numpy as np

import concourse.bass as bass
import concourse.tile as tile
from concourse import bacc, mybir
from concourse.bass_utils import run_bass_kernel_spmd

N = 16384          # neurons
CORES = 8
R = N // CORES     # 2048 rows per core
P = 128            # SBUF partitions
T = R // P         # 16 row-tiles per core; local row = p*T + t
F = 8192           # free-dim chunk size
BF16 = mybir.dt.bfloat16
F32 = mybir.dt.float32

# Per-chunk compute style schedule:
#   "act": DVE tensor_tensor mult + ScalarE activation-accumulate reduce
#   "gps": DVE tensor_tensor mult + GpSimd tensor_scalar-accumulate reduce
#   "dve": DVE tensor_tensor mult + DVE tensor_scalar-accumulate reduce
#   "stt": fused DVE scalar_tensor_tensor multiply-accumulate (one op)
SCHEDULE = ["act", "act", "act", "stt"]


def _style(i):
    return SCHEDULE[i % len(SCHEDULE)]


def build_nc(n=N, r=R, f=F):
    t_tiles = r // P
    k_chunks = n // f
    nc = bacc.Bacc()
    adjb = nc.declare_dram_parameter("adjb", [r, n], BF16, isOutput=False)
    spb = nc.declare_dram_parameter("spb", [P, n], BF16, isOutput=False)
    cx_in = nc.declare_dram_parameter("cx", [2, r], F32, isOutput=False)
    out = nc.declare_dram_parameter("out", [r], F32, isOutput=True)

    adj_t = adjb.rearrange("(p t) n -> t p n", t=t_tiles)   # [T, 128, n]
    cx_t = cx_in.rearrange("v (p t) -> p v t", t=t_tiles)   # [128, 2, T]
    out_t = out.rearrange("(p t) -> p t", t=t_tiles)

    mult = mybir.AluOpType.mult
    add = mybir.AluOpType.add
    bypass = mybir.AluOpType.bypass

    with ExitStack() as ctx:
        tc = ctx.enter_context(tile.TileContext(nc))
        const = ctx.enter_context(tc.tile_pool(name="const", bufs=1))
        loadp = ctx.enter_context(tc.tile_pool(name="load", bufs=3))
        prodp = ctx.enter_context(tc.tile_pool(name="prod", bufs=2))
        sinkp = ctx.enter_context(tc.tile_pool(name="sink", bufs=2))
        partp = ctx.enter_context(tc.tile_pool(name="part", bufs=2))
        smallp = ctx.enter_context(tc.tile_pool(name="small", bufs=1))

        sp_tiles = []
        for k in range(k_chunks):
            spt = const.tile([P, f], BF16, tag=f"sp{k}")
            nc.sync.dma_start(spt[:], spb[:, bass.ts(k, f)])
            sp_tiles.append(spt)
        cx_tile = smallp.tile([P, 2, t_tiles], F32, tag="cx")
        nc.sync.dma_start(cx_tile[:], cx_t[:, :, :])
        d_tile = smallp.tile([P, t_tiles], F32, tag="d")

        # TRN2 allows at most one semaphore wait per instruction; touch each
        # sp tile with a tiny op so the DVE observes those DMA semaphores
        # one at a time before the main loop's tensor_tensor ops.
        touch = smallp.tile([P, 1], BF16, tag="touch")
        for k in range(k_chunks):
            nc.vector.tensor_copy(touch[:], sp_tiles[k][:, 0:1])

        i = 0
        for t in range(t_tiles):
            part = partp.tile([P, k_chunks], F32, tag="part")
            for k in range(k_chunks):
                a = loadp.tile([P, f], BF16, tag="adj")
                nc.sync.dma_start(a[:], adj_t[t][:, bass.ts(k, f)])
                style = _style(i)
                if style == "stt":
                    sink = sinkp.tile([P, f], BF16, tag="sink")
                    nc.vector.scalar_tensor_tensor(
                        sink[:], a[:], 1.0, sp_tiles[k][:],
                        op0=mult, op1=mult,
                        accum_out=part[:, k : k + 1],
                    )
                else:
                    prod = prodp.tile([P, f], BF16, tag="prod")
                    nc.vector.tensor_tensor(prod[:], a[:], sp_tiles[k][:], op=mult)
                    sink = sinkp.tile([P, f], BF16, tag="sink")
                    if style == "dve":
                        nc.vector.tensor_scalar(
                            sink[:], prod[:], 1.0, None,
                            op0=mult, op1=add,
                            accum_out=part[:, k : k + 1],
                        )
                    elif style == "gps":
                        nc.gpsimd.tensor_scalar(
                            sink[:], prod[:], 1.0, None,
                            op0=mult, op1=add,
                            accum_out=part[:, k : k + 1],
                        )
                    else:
                        nc.scalar.activation(
                            sink[:], prod[:],
                            mybir.ActivationFunctionType.Copy,
                            accum_out=part[:, k : k + 1],
                        )
                i += 1
            nc.vector.tensor_reduce(
                d_tile[:, t : t + 1], part[:], axis=mybir.AxisListType.X, op=add
            )

        # Epilogue on [128, T] fp32: out = 1 - (1 - c*x3) * min(d, 1)
        inter = smallp.tile([P, t_tiles], F32, tag="inter")
        nc.vector.tensor_scalar_min(inter[:], d_tile[:], 1.0)
        cn = smallp.tile([P, t_tiles], F32, tag="cn")
        nc.vector.tensor_tensor(cn[:], cx_tile[:, 0, :], cx_tile[:, 1, :], op=mult)
        nc.vector.tensor_scalar(cn[:], cn[:], -1.0, 1.0, op0=mult, op1=add)
        res = smallp.tile([P, t_tiles], F32, tag="res")
        nc.vector.tensor_tensor(res[:], cn[:], inter[:], op=mult)
        nc.vector.tensor_scalar(res[:], res[:], -1.0, 1.0, op0=mult, op1=add)
        nc.sync.dma_start(out_t[:, :], res[:])

    nc.compile()
    return nc


_NC_CACHE = None


def _get_nc():
    global _NC_CACHE
    if _NC_CACHE is None:
        _NC_CACHE = build_nc()
    return _NC_CACHE


def prep_in_maps(x, adj, states, c):
    x = np.asarray(x, dtype=np.float32).reshape(-1)
    adj = np.asarray(adj)
    states = np.asarray(states, dtype=np.float32).reshape(-1)
    c = np.asarray(c, dtype=np.float32).reshape(-1)

    adjb = adj.astype(ml_dtypes.bfloat16)          # exact: adj is 0/1
    sp = (1.0 - states).astype(ml_dtypes.bfloat16)  # exact: states is 0/1
    spb = np.ascontiguousarray(np.broadcast_to(sp[None, :], (P, N)))
    x3 = np.roll(x, -1)                             # x[(i+1) % N]

    in_maps = []
    for m in range(CORES):
        rows = slice(m * R, (m + 1) * R)
        in_maps.append(
            {
                "adjb": np.ascontiguousarray(adjb[rows]),
                "spb": spb,
                "cx": np.ascontiguousarray(np.stack([c[rows], x3[rows]])),
            }
        )
    return in_maps


def _ensure_ntff_hook():
    """Install antenv.axon_hooks shim so trace=True works under axon."""
    import types

    try:
        from antenv.axon_hooks import get_axon_ntff_profile_hook  # noqa: F401

        return
    except ImportError:
        pass
    import antenv
    from trn_agent_boot.trn_boot import _ntff_profile_via_ctypes

    hook = _ntff_profile_via_ctypes("/opt/axon/libaxon_pjrt.so")
    mod = types.ModuleType("antenv.axon_hooks")
    state = {"hook": hook}
    mod.set_axon_ntff_profile_hook = lambda h: state.__setitem__("hook", h)
    mod.get_axon_ntff_profile_hook = lambda: state["hook"]
    sys.modules["antenv.axon_hooks"] = mod
    antenv.axon_hooks = mod


def run(x, adj, states, c, trace=False, **kw):
    if trace:
        _ensure_ntff_hook()
    in_maps = prep_in_maps(x, adj, states, c)
    res = run_bass_kernel_spmd(
        _get_nc(), in_maps, list(range(CORES)), trace=trace, **kw
    )
    outs = [np.asarray(res.results[m]["out"], dtype=np.float32) for m in range(CORES)]
    full = np.concatenate([o.reshape(R) for o in outs])
    return full, res


def kernel(x, adj, states, c):
    full, _ = run(x, adj, states, c)
    return full


# revision 13
# speedup vs baseline: 1.3078x; 1.0892x over previous
"""Trainium2 Bass kernel for nn_BITypeNetwork (16384-neuron BI-type network step).

Math: the reference computes, with adj/states exactly binary {0.0, 1.0},
    inter_i = 1 - prod_j (1 - adj[i,j] + adj[i,j]*states[j])
Each product term equals 1 - adj[i,j]*(1 - states[j]) which is 0 or 1, so
    inter_i = min(sum_j adj[i,j] * (1 - states[j]), 1)
i.e. a matvec against sp = 1 - states followed by a clamp — exact in fp32
(and the 0/1 operands are exact in bf16, halving the HBM traffic).
Tail:  out = 1 - (1 - c * roll(x, -1)) * inter.

Sharding: adj row-sharded across 8 cores (2048 rows each), sp broadcast,
c/x3 row-sharded. No cross-device reduction needed.

Per core the kernel streams its [2048, 16384] adj shard as bf16 through
DVE tensor_tensor multiply (2x_1p mode) with per-chunk row-sums taken by
ScalarE activation-accumulate / DVE tensor_scalar-accumulate (4x mode),
so both compute engines stay under the ~190us DMA roofline.
"""

import os
import sys

for _p in ("/opt/trn_rl_repo", "/opt/pypackages"):
    if os.path.isdir(_p) and _p not in sys.path:
        sys.path.insert(0, _p)

from contextlib import ExitStack

import ml_dtypes
import numpy as np

import concourse.bass as bass
import concourse.tile as tile
from concourse import bacc, mybir
from concourse.bass_utils import run_bass_kernel_spmd

N = 16384          # neurons
CORES = 8
R = N // CORES     # 2048 rows per core
P = 128            # SBUF partitions
T = R // P         # 16 row-tiles per core; local row = p*T + t
F = 8192           # free-dim chunk size
BF16 = mybir.dt.bfloat16
F32 = mybir.dt.float32

# Per-chunk compute style schedule:
#   "act": DVE tensor_tensor mult + ScalarE activation-accumulate reduce
#   "gps": DVE tensor_tensor mult + GpSimd tensor_scalar-accumulate reduce
#   "dve": DVE tensor_tensor mult + DVE tensor_scalar-accumulate reduce
#   "stt": fused DVE scalar_tensor_tensor multiply-accumulate (one op)
SCHEDULE = ["act", "act", "act", "stt"]


def _style(i):
    return SCHEDULE[i % len(SCHEDULE)]


def build_nc(n=N, r=R, f=F):
    t_tiles = r // P
    k_chunks = n // f
    nc = bacc.Bacc()
    adjb = nc.declare_dram_parameter("adjb", [r, n], BF16, isOutput=False)
    spb = nc.declare_dram_parameter("spb", [P, n], BF16, isOutput=False)
    cx_in = nc.declare_dram_parameter("cx", [2, r], F32, isOutput=False)
    out = nc.declare_dram_parameter("out", [r], F32, isOutput=True)

    adj_t = adjb.rearrange("(p t) n -> t p n", t=t_tiles)   # [T, 128, n]
    cx_t = cx_in.rearrange("v (p t) -> p v t", t=t_tiles)   # [128, 2, T]
    out_t = out.rearrange("(p t) -> p t", t=t_tiles)

    mult = mybir.AluOpType.mult
    add = mybir.AluOpType.add
    bypass = mybir.AluOpType.bypass

    with ExitStack() as ctx:
        tc = ctx.enter_context(tile.TileContext(nc))
        const = ctx.enter_context(tc.tile_pool(name="const", bufs=1))
        loadp = ctx.enter_context(tc.tile_pool(name="load", bufs=4))
        prodp = ctx.enter_context(tc.tile_pool(name="prod", bufs=2))
        sinkp = ctx.enter_context(tc.tile_pool(name="sink", bufs=3))
        partp = ctx.enter_context(tc.tile_pool(name="part", bufs=2))
        smallp = ctx.enter_context(tc.tile_pool(name="small", bufs=1))

        sp_tiles = []
        for k in range(k_chunks):
            spt = const.tile([P, f], BF16, tag=f"sp{k}")
            nc.sync.dma_start(spt[:], spb[:, bass.ts(k, f)])
            sp_tiles.append(spt)
        cx_tile = smallp.tile([P, 2, t_tiles], F32, tag="cx")
        nc.sync.dma_start(cx_tile[:], cx_t[:, :, :])
        d_tile = smallp.tile([P, t_tiles], F32, tag="d")

        # TRN2 allows at most one semaphore wait per instruction; touch each
        # sp tile with a tiny op so the DVE observes those DMA semaphores
        # one at a time before the main loop's tensor_tensor ops.
        touch = smallp.tile([P, 1], BF16, tag="touch")
        for k in range(k_chunks):
            nc.vector.tensor_copy(touch[:], sp_tiles[k][:, 0:1])

        i = 0
        for t in range(t_tiles):
            part = partp.tile([P, k_chunks], F32, tag="part")
            for k in range(k_chunks):
                a = loadp.tile([P, f], BF16, tag="adj")
                nc.sync.dma_start(a[:], adj_t[t][:, bass.ts(k, f)])
                style = _style(i)
                if style == "stt":
                    sink = sinkp.tile([P, f], BF16, tag="sink")
                    nc.vector.scalar_tensor_tensor(
                        sink[:], a[:], 1.0, sp_tiles[k][:],
                        op0=mult, op1=mult,
                        accum_out=part[:, k : k + 1],
                    )
                else:
                    prod = prodp.tile([P, f], BF16, tag="prod")
                    nc.vector.tensor_tensor(prod[:], a[:], sp_tiles[k][:], op=mult)
                    sink = sinkp.tile([P, f], BF16, tag="sink")
                    if style == "dve":
                        nc.vector.tensor_scalar(
                            sink[:], prod[:], 1.0, None,
                            op0=mult, op1=add,
                            accum_out=part[:, k : k + 1],
                        )
                    elif style == "gps":
                        nc.gpsimd.tensor_scalar(
                            sink[:], prod[:], 1.0, None,
                            op0=mult, op1=add,
                            accum_out=part[:, k : k + 1],
                        )
                    else:
                        nc.scalar.activation(
                            sink[:], prod[:],
                            mybir.ActivationFunctionType.Copy,
                            accum_out=part[:, k : k + 1],
                        )
                i += 1
            nc.vector.tensor_reduce(
                d_tile[:, t : t + 1], part[:], axis=mybir.AxisListType.X, op=add
            )

        # Epilogue on [128, T] fp32: out = 1 - (1 - c*x3) * min(d, 1)
        inter = smallp.tile([P, t_tiles], F32, tag="inter")
        nc.vector.tensor_scalar_min(inter[:], d_tile[:], 1.0)
        cn = smallp.tile([P, t_tiles], F32, tag="cn")
        nc.vector.tensor_tensor(cn[:], cx_tile[:, 0, :], cx_tile[:, 1, :], op=mult)
        nc.vector.tensor_scalar(cn[:], cn[:], -1.0, 1.0, op0=mult, op1=add)
        res = smallp.tile([P, t_tiles], F32, tag="res")
        nc.vector.tensor_tensor(res[:], cn[:], inter[:], op=mult)
        nc.vector.tensor_scalar(res[:], res[:], -1.0, 1.0, op0=mult, op1=add)
        nc.sync.dma_start(out_t[:, :], res[:])

    nc.compile()
    return nc


_NC_CACHE = None


def _get_nc():
    global _NC_CACHE
    if _NC_CACHE is None:
        _NC_CACHE = build_nc()
    return _NC_CACHE


def prep_in_maps(x, adj, states, c):
    x = np.asarray(x, dtype=np.float32).reshape(-1)
    adj = np.asarray(adj)
    states = np.asarray(states, dtype=np.float32).reshape(-1)
    c = np.asarray(c, dtype=np.float32).reshape(-1)

    adjb = adj.astype(ml_dtypes.bfloat16)          # exact: adj is 0/1
    sp = (1.0 - states).astype(ml_dtypes.bfloat16)  # exact: states is 0/1
    spb = np.ascontiguousarray(np.broadcast_to(sp[None, :], (P, N)))
    x3 = np.roll(x, -1)                             # x[(i+1) % N]

    in_maps = []
    for m in range(CORES):
        rows = slice(m * R, (m + 1) * R)
        in_maps.append(
            {
                "adjb": np.ascontiguousarray(adjb[rows]),
                "spb": spb,
                "cx": np.ascontiguousarray(np.stack([c[rows], x3[rows]])),
            }
        )
    return in_maps


def _ensure_ntff_hook():
    """Install antenv.axon_hooks shim so trace=True works under axon."""
    import types

    try:
        from antenv.axon_hooks import get_axon_ntff_profile_hook  # noqa: F401

        return
    except ImportError:
        pass
    import antenv
    from trn_agent_boot.trn_boot import _ntff_profile_via_ctypes

    hook = _ntff_profile_via_ctypes("/opt/axon/libaxon_pjrt.so")
    mod = types.ModuleType("antenv.axon_hooks")
    state = {"hook": hook}
    mod.set_axon_ntff_profile_hook = lambda h: state.__setitem__("hook", h)
    mod.get_axon_ntff_profile_hook = lambda: state["hook"]
    sys.modules["antenv.axon_hooks"] = mod
    antenv.axon_hooks = mod


def run(x, adj, states, c, trace=False, **kw):
    if trace:
        _ensure_ntff_hook()
    in_maps = prep_in_maps(x, adj, states, c)
    res = run_bass_kernel_spmd(
        _get_nc(), in_maps, list(range(CORES)), trace=trace, **kw
    )
    outs = [np.asarray(res.results[m]["out"], dtype=np.float32) for m in range(CORES)]
    full = np.concatenate([o.reshape(R) for o in outs])
    return full, res


def kernel(x, adj, states, c):
    full, _ = run(x, adj, states, c)
    return full


# revision 14
# speedup vs baseline: 1.3229x; 1.0115x over previous
"""Trainium2 Bass kernel for nn_BITypeNetwork (16384-neuron BI-type network step).

Math: the reference computes, with adj/states exactly binary {0.0, 1.0},
    inter_i = 1 - prod_j (1 - adj[i,j] + adj[i,j]*states[j])
Each product term equals 1 - adj[i,j]*(1 - states[j]) which is 0 or 1, so
    inter_i = min(sum_j adj[i,j] * (1 - states[j]), 1)
i.e. a matvec against sp = 1 - states followed by a clamp — exact in fp32
(and the 0/1 operands are exact in bf16, halving the HBM traffic).
Tail:  out = 1 - (1 - c * roll(x, -1)) * inter.

Sharding: adj row-sharded across 8 cores (2048 rows each), sp broadcast,
c/x3 row-sharded. No cross-device reduction needed.

Per core the kernel streams its [2048, 16384] adj shard as bf16 through
DVE tensor_tensor multiply (2x_1p mode) with per-chunk row-sums taken by
ScalarE activation-accumulate / DVE tensor_scalar-accumulate (4x mode),
so both compute engines stay under the ~190us DMA roofline.
"""

import os
import sys

for _p in ("/opt/trn_rl_repo", "/opt/pypackages"):
    if os.path.isdir(_p) and _p not in sys.path:
        sys.path.insert(0, _p)

from contextlib import ExitStack

import ml_dtypes
import numpy as np

import concourse.bass as bass
import concourse.tile as tile
from concourse import bacc, mybir
from concourse.bass_utils import run_bass_kernel_spmd

N = 16384          # neurons
CORES = 8
R = N // CORES     # 2048 rows per core
P = 128            # SBUF partitions
T = R // P         # 16 row-tiles per core; local row = p*T + t
F = 8192           # free-dim chunk size
BF16 = mybir.dt.bfloat16
F32 = mybir.dt.float32

# Per-chunk compute style schedule:
#   "act": DVE tensor_tensor mult + ScalarE activation-accumulate reduce
#   "gps": DVE tensor_tensor mult + GpSimd tensor_scalar-accumulate reduce
#   "dve": DVE tensor_tensor mult + DVE tensor_scalar-accumulate reduce
#   "stt": fused DVE scalar_tensor_tensor multiply-accumulate (one op)
SCHEDULE = ["stt" if (i * 9) // 32 != ((i + 1) * 9) // 32 else "act" for i in range(32)]


def _style(i):
    return SCHEDULE[i % len(SCHEDULE)]


def build_nc(n=N, r=R, f=F):
    t_tiles = r // P
    k_chunks = n // f
    nc = bacc.Bacc()
    adjb = nc.declare_dram_parameter("adjb", [r, n], BF16, isOutput=False)
    spb = nc.declare_dram_parameter("spb", [P, n], BF16, isOutput=False)
    cx_in = nc.declare_dram_parameter("cx", [2, r], F32, isOutput=False)
    out = nc.declare_dram_parameter("out", [r], F32, isOutput=True)

    adj_t = adjb.rearrange("(p t) n -> t p n", t=t_tiles)   # [T, 128, n]
    cx_t = cx_in.rearrange("v (p t) -> p v t", t=t_tiles)   # [128, 2, T]
    out_t = out.rearrange("(p t) -> p t", t=t_tiles)

    mult = mybir.AluOpType.mult
    add = mybir.AluOpType.add
    bypass = mybir.AluOpType.bypass

    with ExitStack() as ctx:
        tc = ctx.enter_context(tile.TileContext(nc))
        const = ctx.enter_context(tc.tile_pool(name="const", bufs=1))
        loadp = ctx.enter_context(tc.tile_pool(name="load", bufs=4))
        prodp = ctx.enter_context(tc.tile_pool(name="prod", bufs=2))
        sinkp = ctx.enter_context(tc.tile_pool(name="sink", bufs=3))
        partp = ctx.enter_context(tc.tile_pool(name="part", bufs=2))
        smallp = ctx.enter_context(tc.tile_pool(name="small", bufs=1))

        sp_tiles = []
        for k in range(k_chunks):
            spt = const.tile([P, f], BF16, tag=f"sp{k}")
            nc.sync.dma_start(spt[:], spb[:, bass.ts(k, f)])
            sp_tiles.append(spt)
        cx_tile = smallp.tile([P, 2, t_tiles], F32, tag="cx")
        nc.sync.dma_start(cx_tile[:], cx_t[:, :, :])
        d_tile = smallp.tile([P, t_tiles], F32, tag="d")

        # TRN2 allows at most one semaphore wait per instruction; touch each
        # sp tile with a tiny op so the DVE observes those DMA semaphores
        # one at a time before the main loop's tensor_tensor ops.
        touch = smallp.tile([P, 1], BF16, tag="touch")
        for k in range(k_chunks):
            nc.vector.tensor_copy(touch[:], sp_tiles[k][:, 0:1])

        i = 0
        for t in range(t_tiles):
            part = partp.tile([P, k_chunks], F32, tag="part")
            for k in range(k_chunks):
                a = loadp.tile([P, f], BF16, tag="adj")
                nc.sync.dma_start(a[:], adj_t[t][:, bass.ts(k, f)])
                style = _style(i)
                if style == "stt":
                    sink = sinkp.tile([P, f], BF16, tag="sink")
                    nc.vector.scalar_tensor_tensor(
                        sink[:], a[:], 1.0, sp_tiles[k][:],
                        op0=mult, op1=mult,
                        accum_out=part[:, k : k + 1],
                    )
                else:
                    prod = prodp.tile([P, f], BF16, tag="prod")
                    nc.vector.tensor_tensor(prod[:], a[:], sp_tiles[k][:], op=mult)
                    sink = sinkp.tile([P, f], BF16, tag="sink")
                    if style == "dve":
                        nc.vector.tensor_scalar(
                            sink[:], prod[:], 1.0, None,
                            op0=mult, op1=add,
                            accum_out=part[:, k : k + 1],
                        )
                    elif style == "gps":
                        nc.gpsimd.tensor_scalar(
                            sink[:], prod[:], 1.0, None,
                            op0=mult, op1=add,
                            accum_out=part[:, k : k + 1],
                        )
                    else:
                        nc.scalar.activation(
                            sink[:], prod[:],
                            mybir.ActivationFunctionType.Copy,
                            accum_out=part[:, k : k + 1],
                        )
                i += 1
            nc.vector.tensor_reduce(
                d_tile[:, t : t + 1], part[:], axis=mybir.AxisListType.X, op=add
            )

        # Epilogue on [128, T] fp32: out = 1 - (1 - c*x3) * min(d, 1)
        inter = smallp.tile([P, t_tiles], F32, tag="inter")
        nc.vector.tensor_scalar_min(inter[:], d_tile[:], 1.0)
        cn = smallp.tile([P, t_tiles], F32, tag="cn")
        nc.vector.tensor_tensor(cn[:], cx_tile[:, 0, :], cx_tile[:, 1, :], op=mult)
        nc.vector.tensor_scalar(cn[:], cn[:], -1.0, 1.0, op0=mult, op1=add)
        res = smallp.tile([P, t_tiles], F32, tag="res")
        nc.vector.tensor_tensor(res[:], cn[:], inter[:], op=mult)
        nc.vector.tensor_scalar(res[:], res[:], -1.0, 1.0, op0=mult, op1=add)
        nc.sync.dma_start(out_t[:, :], res[:])

    nc.compile()
    return nc


_NC_CACHE = None


def _get_nc():
    global _NC_CACHE
    if _NC_CACHE is None:
        _NC_CACHE = build_nc()
    return _NC_CACHE


def prep_in_maps(x, adj, states, c):
    x = np.asarray(x, dtype=np.float32).reshape(-1)
    adj = np.asarray(adj)
    states = np.asarray(states, dtype=np.float32).reshape(-1)
    c = np.asarray(c, dtype=np.float32).reshape(-1)

    adjb = adj.astype(ml_dtypes.bfloat16)          # exact: adj is 0/1
    sp = (1.0 - states).astype(ml_dtypes.bfloat16)  # exact: states is 0/1
    spb = np.ascontiguousarray(np.broadcast_to(sp[None, :], (P, N)))
    x3 = np.roll(x, -1)                             # x[(i+1) % N]

    in_maps = []
    for m in range(CORES):
        rows = slice(m * R, (m + 1) * R)
        in_maps.append(
            {
                "adjb": np.ascontiguousarray(adjb[rows]),
                "spb": spb,
                "cx": np.ascontiguousarray(np.stack([c[rows], x3[rows]])),
            }
        )
    return in_maps


def _ensure_ntff_hook():
    """Install antenv.axon_hooks shim so trace=True works under axon."""
    import types

    try:
        from antenv.axon_hooks import get_axon_ntff_profile_hook  # noqa: F401

        return
    except ImportError:
        pass
    import antenv
    from trn_agent_boot.trn_boot import _ntff_profile_via_ctypes

    hook = _ntff_profile_via_ctypes("/opt/axon/libaxon_pjrt.so")
    mod = types.ModuleType("antenv.axon_hooks")
    state = {"hook": hook}
    mod.set_axon_ntff_profile_hook = lambda h: state.__setitem__("hook", h)
    mod.get_axon_ntff_profile_hook = lambda: state["hook"]
    sys.modules["antenv.axon_hooks"] = mod
    antenv.axon_hooks = mod


def run(x, adj, states, c, trace=False, **kw):
    if trace:
        _ensure_ntff_hook()
    in_maps = prep_in_maps(x, adj, states, c)
    res = run_bass_kernel_spmd(
        _get_nc(), in_maps, list(range(CORES)), trace=trace, **kw
    )
    outs = [np.asarray(res.results[m]["out"], dtype=np.float32) for m in range(CORES)]
    full = np.concatenate([o.reshape(R) for o in outs])
    return full, res


def kernel(x, adj, states, c):
    full, _ = run(x, adj, states, c)
    return full
